# revision 9
# baseline (speedup 1.0000x reference)
"""BirthDeathIntervalLoss on 8 Trainium2 NeuronCores.

Strategy: the loss reads only 2*B*C*N*2 = 32768 scattered elements of the
512x512 prediction maps.  Data-parallel over batch: each core handles 4
batches (4096 gathered values).  Per core the device program is minimal:

  1. one HWDGE DMA brings a packed [128, 48] i32 tile into SBUF
     (cols 0:32 = host-precomputed flat gather indices, cols 32:48 =
     per-pair weights bit-cast to i32),
  2. ONE indirect DMA (SWDGE) gathers all 4096 prediction values into a
     single 16KB partition row (the previous version split this into 8
     serial calls at ~1.75us fixed cost each; one call pays the ~1us SWDGE
     fixed overhead once and the ring holds 16x scratch/16 descriptors).
     A gather call can only write one partition row: the descriptor count
     equals the dest AP's free size, and offsets are consumed
     partition-fastest (HW-calibrated; CoreSim's ravel order differs),
  3. one direct SBUF->SBUF DMA reshapes the row to [128, 32] so the
     vector engine gets all 128 lanes,
  4. the vector engine computes w*(birth-death)^2 in [128, 16] layout and
     reduces along the free axis to [128, 1] partials,
  5. one HWDGE DMA writes the 512-byte partials out.

The host sums 128 partials x 8 cores and adds the closed-form constant
(this is the data-parallel all-reduce of the scalar loss).

Masked-mean algebra (unchanged from the reference):
  loss = sum_m w_m * (birth_m - death_m)^2 + B * sum_s a_s*BETA*cnt_s / C
  w_m  = a_s * (-BETA/good_s[c] if n < good_s[c] else (1-BETA)/(N-good_s[c])) / C
with a_0 = ALPHA, a_1 = 1-ALPHA, cnt_s = #{c : good_s[c] > 0}.

Value placement: pair m (natural (set,batch,class,n) order) sits at
partition p = m % 128, column q = m // 128 (q < 16); after the reshape its
birth is at gsq[p, q], death at gsq[p, q + 16], weight at wts[p, q].
Descriptor j writes grow[0, j] and reads offset slot idx[j % 128, j // 128];
the reshape maps grow[0, 32p + f] -> gsq[p, f], so the birth of pair (p, q)
is descriptor j = 32p + q and the death is j = 32p + 16 + q.
"""

import numpy as np

# ---- problem constants (hardcoded per harness contract) ----
B, C, H, W, N = 32, 4, 512, 512, 64
GOOD = np.array([[1, 2, 1, 3], [1, 0, 2, 1]], dtype=np.int64)  # [set, class]
ALPHA = 0.5
BETA = 0.5
N_CORES = 8
B_LOC = B // N_CORES  # 4 batches per core

PRED_LOC = B_LOC * C * H * W          # 4,194,304 f32 per core
N_PAIRS = 2 * B_LOC * C * N           # 2048 (birth,death) pairs per core
N_VALS = 2 * N_PAIRS                  # 4096 gathered values per core

P = 128                               # partitions
QCOL = N_PAIRS // P                   # 16 pair columns
FV = 2 * QCOL                         # 32 gathered-value columns
FI = FV + QCOL                        # 48 input columns (idx + weights)


def _host_constants():
    """Natural-order pair weights w[m] and the per-core additive constant."""
    a = np.array([ALPHA, 1.0 - ALPHA])
    m = np.arange(N_PAIRS)
    s = m // (B_LOC * C * N)
    cc = (m // N) % C
    n = m % N
    g = GOOD[s, cc]
    w = np.where(
        n < g,
        -a[s] * BETA / np.maximum(g, 1) / C,
        a[s] * (1.0 - BETA) / (N - g) / C,
    ).astype(np.float32)
    cnt = (GOOD > 0).sum(axis=1)  # per set
    const_per_core = float((a * BETA * cnt / C).sum() * B_LOC)
    return w, const_per_core


_W_NAT, _CONST = _host_constants()

# pair m -> (partition p, pair column q)
_MP = np.arange(N_PAIRS) % P
_MQ = np.arange(N_PAIRS) // P

# weights tile [P, QCOL]
_WTS = np.zeros((P, QCOL), dtype=np.float32)
_WTS[_MP, _MQ] = _W_NAT

# descriptor ids for pair m's birth/death, and the offset slot of desc j
# (offsets are consumed partition-fastest: desc j <- idx[j % P, j // P])
_JB = _MP * FV + _MQ
_JD = _MP * FV + QCOL + _MQ

# per-value image base, natural value order (u = 2*m + endpoint? no: values
# ordered (s,b,c,n) x endpoint via the intervals axis handled in kernel())
_MB = (np.arange(N_PAIRS) // (C * N)) % B_LOC
_MC = (np.arange(N_PAIRS) // N) % C
_IMGBASE = ((_MB * C + _MC) * (H * W)).astype(np.int64)  # per pair

_PROGRAM = None
_LAST_RESULTS = None  # BassKernelResults of the most recent run (for test.py)
TRACE = False
DEBUG_G = False  # build with an extra DMA dumping the gathered tile


def _build_program(debug_g=False):
    from concourse import bacc, mybir
    import concourse.bass as bass
    import concourse.tile as tile

    f32 = mybir.dt.float32
    i32 = mybir.dt.int32

    # default 16KB SWDGE scratch holds only 1024 16-byte descriptors; the
    # single 4096-descriptor gather needs a bigger ring (64KB + slack).
    nc = bacc.Bacc(
        "TRN2",
        target_bir_lowering=False,
        debug=False,
        dynamic_dma_scratch_size=131072,
    )

    pred_d = nc.dram_tensor("pred", [PRED_LOC], f32, kind="ExternalInput")
    inp_d = nc.dram_tensor("inp", [P, FI], i32, kind="ExternalInput")
    out_d = nc.dram_tensor("out", [P, 1], f32, kind="ExternalOutput")
    if debug_g:
        g_d = nc.dram_tensor("gdump", [P, FV], f32, kind="ExternalOutput")

    with tile.TileContext(nc) as tc:
        with tc.tile_pool(name="sb", bufs=1) as pool:
            inp = pool.tile([P, FI], i32)
            nc.sync.dma_start(inp[:], inp_d[:])

            # ONE indirect DMA: 4096 descriptors, one 4-byte value each,
            # all landing in a single partition row.
            grow = pool.tile([1, N_VALS], f32)
            src = pred_d.ap().rearrange("(a f) -> a f", a=1)
            nc.gpsimd.indirect_dma_start(
                out=grow[:].rearrange("a (f one) -> a f one", one=1),
                out_offset=None,
                in_=src,
                in_offset=bass.IndirectOffsetOnAxis(ap=inp[:, 0:FV], axis=1),
            )

            # reshape the row across all 128 partitions: gsq[p, f] = grow[0, 32p+f]
            # (src AP stays 3D [1, 128, 32] — merging the 128-chunk dim into
            # the partition axis would make the DMA read partitions 1..127)
            gsq = pool.tile([P, FV], f32)
            nc.sync.dma_start(
                gsq[:], grow[:].rearrange("a (p f) -> a p f", p=P)
            )
            if debug_g:
                nc.scalar.dma_start(g_d[:], gsq[:])

            wv = inp[:, FV:FI].bitcast(f32)
            d = pool.tile([P, QCOL], f32)
            nc.vector.tensor_tensor(
                out=d[:], in0=gsq[:, 0:QCOL], in1=gsq[:, QCOL:FV],
                op=mybir.AluOpType.subtract,
            )
            d2 = pool.tile([P, QCOL], f32)
            nc.vector.tensor_tensor(
                out=d2[:], in0=d[:], in1=d[:], op=mybir.AluOpType.mult
            )
            dw = pool.tile([P, QCOL], f32)
            nc.vector.tensor_tensor(
                out=dw[:], in0=d2[:], in1=wv, op=mybir.AluOpType.mult
            )
            r = pool.tile([P, 1], f32)
            nc.vector.reduce_sum(out=r[:], in_=dw[:], axis=mybir.AxisListType.X)
            nc.sync.dma_start(out_d[:], r[:])

    nc.compile()
    return nc


def _get_program():
    global _PROGRAM
    if _PROGRAM is None:
        _PROGRAM = _build_program(debug_g=DEBUG_G)
    return _PROGRAM


def _pack_core(i0_sl, i1_sl):
    """Build the [P, FI] i32 input tile for one core's interval tensors."""
    iv = np.stack([i0_sl, i1_sl])  # [2, B_LOC, C, N, 2, 2]
    r_b = iv[..., 0, 0].reshape(-1).astype(np.int64)
    c_b = iv[..., 0, 1].reshape(-1).astype(np.int64)
    r_d = iv[..., 1, 0].reshape(-1).astype(np.int64)
    c_d = iv[..., 1, 1].reshape(-1).astype(np.int64)
    fb = (_IMGBASE + r_b * W + c_b).astype(np.int32)
    fd = (_IMGBASE + r_d * W + c_d).astype(np.int32)
    idx = np.empty((P, FV), dtype=np.int32)
    idx[_JB % P, _JB // P] = fb
    idx[_JD % P, _JD // P] = fd
    inp = np.empty((P, FI), dtype=np.int32)
    inp[:, :FV] = idx
    inp[:, FV:] = _WTS.view(np.int32)
    return inp


def kernel(prediction, intervals_comp_0, intervals_comp_1):
    global _LAST_RESULTS
    from concourse.bass_utils import run_bass_kernel_spmd

    nc = _get_program()

    prediction = np.asarray(prediction, dtype=np.float32)
    i0 = np.asarray(intervals_comp_0, dtype=np.int32)
    i1 = np.asarray(intervals_comp_1, dtype=np.int32)

    in_maps = []
    for mcore in range(N_CORES):
        sl = slice(mcore * B_LOC, (mcore + 1) * B_LOC)
        in_maps.append(
            {
                "pred": np.ascontiguousarray(prediction[sl]).reshape(-1),
                "inp": _pack_core(i0[sl], i1[sl]),
            }
        )

    results = run_bass_kernel_spmd(
        nc, in_maps, list(range(N_CORES)), trace=TRACE
    )
    _LAST_RESULTS = results
    total = sum(float(r["out"].sum()) for r in results.results)
    total += N_CORES * _CONST
    return np.array(total, dtype=np.float32)


# revision 14
# speedup vs baseline: 1.4316x; 1.4316x over previous
"""BirthDeathIntervalLoss on 8 Trainium2 NeuronCores.

Strategy: the loss reads only 2*B*C*N*2 = 32768 scattered elements of the
512x512 prediction maps.  Data-parallel over batch: each core handles 4
batches (4096 gathered values).  Per core the device program is:

  1. one HWDGE DMA brings host-precomputed flat gather indices
     ([128, 32] i32) into SBUF; the per-pair weights ([NCALLS, PAIRS_ROW]
     f32) ride the scalar engine's ring in parallel,
  2. NCALLS indirect DMAs (SWDGE q0) gather the prediction values row by
     row.  The SWDGE ring retires descriptors serially at ~4ns each
     (measured; the data transfers themselves batch 64 payloads per
     packet and finish in ~1.5us), so the ~17us wall is set by 4096
     descriptors regardless of call structure; splitting into several
     calls lets descriptor generation and earlier calls' retirement
     pipeline, and a small final call shortens the completion tail.
     A gather call can only write ONE partition row (descriptor count =
     dest free size; offsets are consumed partition-fastest).
  3. the vector engine computes w*(birth-death)^2 on the [NCALLS, .] rows
     (births in the first half of each row, deaths in the second half, so
     every DVE operand is unit-stride), reduces along the free axis,
  4. the PE collapses the NCALLS partitions via ones^T matmul and one
     HWDGE DMA writes the 4-byte scalar out ([128,1]-style outputs cost
     ~8us of completion-semaphore crawl).

The host sums 8 per-core scalars and adds the closed-form constant
(this is the data-parallel all-reduce of the scalar loss).

Masked-mean algebra (unchanged from the reference):
  loss = sum_m w_m * (birth_m - death_m)^2 + B * sum_s a_s*BETA*cnt_s / C
  w_m  = a_s * (-BETA/good_s[c] if n < good_s[c] else (1-BETA)/(N-good_s[c])) / C
with a_0 = ALPHA, a_1 = 1-ALPHA, cnt_s = #{c : good_s[c] > 0}.

Descriptor walk (HW-calibrated): call c consumes offset columns
[c*CPC, (c+1)*CPC) of the [128, 32] idx tile partition-fastest — desc j
reads idx[j % 128, c*CPC + j // 128] — and writes g[c, j].
"""

import numpy as np

# ---- problem constants (hardcoded per harness contract) ----
B, C, H, W, N = 32, 4, 512, 512, 64
GOOD = np.array([[1, 2, 1, 3], [1, 0, 2, 1]], dtype=np.int64)  # [set, class]
ALPHA = 0.5
BETA = 0.5
N_CORES = 8
B_LOC = B // N_CORES  # 4 batches per core

PRED_LOC = B_LOC * C * H * W          # 4,194,304 f32 per core
N_PAIRS = 2 * B_LOC * C * N           # 2048 (birth,death) pairs per core
N_VALS = 2 * N_PAIRS                  # 4096 gathered values per core

P = 128                               # partitions
FV = N_VALS // P                      # 32 offset columns

NCALLS = 8                            # gather calls (rows of g)
CPC = FV // NCALLS                    # offset columns per call
ROWLEN = N_VALS // NCALLS             # values per row
HALF = ROWLEN // 2                    # pairs per row
SCRATCH = 16384                       # SWDGE descriptor ring bytes


def _host_constants():
    """Natural-order pair weights w[m] and the per-core additive constant."""
    a = np.array([ALPHA, 1.0 - ALPHA])
    m = np.arange(N_PAIRS)
    s = m // (B_LOC * C * N)
    cc = (m // N) % C
    n = m % N
    g = GOOD[s, cc]
    w = np.where(
        n < g,
        -a[s] * BETA / np.maximum(g, 1) / C,
        a[s] * (1.0 - BETA) / (N - g) / C,
    ).astype(np.float32)
    cnt = (GOOD > 0).sum(axis=1)  # per set
    const_per_core = float((a * BETA * cnt / C).sum() * B_LOC)
    return w, const_per_core


_W_NAT, _CONST = _host_constants()

# pair m -> (call c, pair slot q); birth desc j=q, death desc j=HALF+q of call c
_MC = np.arange(N_PAIRS) // HALF
_MQ = np.arange(N_PAIRS) % HALF

_WTS = _W_NAT.reshape(NCALLS, HALF).copy()  # wts[c, q] = w of pair c*HALF+q

# offset slot of desc j in call c: idx[j % P, c*CPC + j // P]
_JB = _MQ            # birth desc id within call
_JD = HALF + _MQ     # death desc id within call
_IB_P = _JB % P
_IB_F = _MC * CPC + _JB // P
_ID_P = _JD % P
_ID_F = _MC * CPC + _JD // P

# per-pair image base
_MB = (np.arange(N_PAIRS) // (C * N)) % B_LOC
_MCC = (np.arange(N_PAIRS) // N) % C
_IMGBASE = ((_MB * C + _MCC) * (H * W)).astype(np.int64)  # per pair

_PROGRAM = None
_LAST_RESULTS = None  # BassKernelResults of the most recent run (for test.py)
TRACE = False
DEBUG_G = False  # build with an extra DMA dumping the gathered rows


def _build_program(debug_g=False):
    from concourse import bacc, mybir
    import concourse.bass as bass
    import concourse.tile as tile

    f32 = mybir.dt.float32
    i32 = mybir.dt.int32

    nc = bacc.Bacc(
        "TRN2",
        target_bir_lowering=False,
        debug=False,
        dynamic_dma_scratch_size=SCRATCH,
    )

    pred_d = nc.dram_tensor("pred", [PRED_LOC], f32, kind="ExternalInput")
    idx_d = nc.dram_tensor("idx", [P, FV], i32, kind="ExternalInput")
    wts_d = nc.dram_tensor("wts", [NCALLS, HALF], f32, kind="ExternalInput")
    out_d = nc.dram_tensor("out", [1, 1], f32, kind="ExternalOutput")
    if debug_g:
        g_d = nc.dram_tensor("gdump", [NCALLS, ROWLEN], f32, kind="ExternalOutput")

    with tile.TileContext(nc) as tc:
        with (
            tc.tile_pool(name="sb", bufs=1) as pool,
            tc.tile_pool(name="ps", bufs=1, space="PSUM") as psp,
        ):
            # idx rides the gpsimd SWDGE ring: gpsimd's preamble ends ~1us
            # before sync's (sync runs the startup DRAIN), and the gather
            # gens queue on the same engine right behind it
            idx = pool.tile([P, FV], i32)
            nc.gpsimd.dma_start(idx[:], idx_d[:])
            wts = pool.tile([NCALLS, HALF], f32)
            nc.scalar.dma_start(wts[:], wts_d[:])
            ones = pool.tile([NCALLS, 1], f32)
            nc.vector.memset(ones[:], 1.0)

            g = pool.tile([NCALLS, ROWLEN], f32)
            src = pred_d.ap().rearrange("(a f) -> a f", a=1)
            for c in range(NCALLS):
                nc.gpsimd.indirect_dma_start(
                    out=g[c : c + 1, :].rearrange("a (f one) -> a f one", one=1),
                    out_offset=None,
                    in_=src,
                    in_offset=bass.IndirectOffsetOnAxis(
                        ap=idx[:, c * CPC : (c + 1) * CPC], axis=1
                    ),
                )
            if debug_g:
                nc.scalar.dma_start(g_d[:], g[:])

            d = pool.tile([NCALLS, HALF], f32)
            nc.vector.tensor_tensor(
                out=d[:], in0=g[:, 0:HALF], in1=g[:, HALF:ROWLEN],
                op=mybir.AluOpType.subtract,
            )
            d2 = pool.tile([NCALLS, HALF], f32)
            nc.vector.tensor_tensor(
                out=d2[:], in0=d[:], in1=d[:], op=mybir.AluOpType.mult
            )
            dw = pool.tile([NCALLS, HALF], f32)
            nc.vector.tensor_tensor(
                out=dw[:], in0=d2[:], in1=wts[:], op=mybir.AluOpType.mult
            )
            r = pool.tile([NCALLS, 1], f32)
            nc.vector.reduce_sum(out=r[:], in_=dw[:], axis=mybir.AxisListType.X)
            acc = psp.tile([1, 1], f32)
            nc.tensor.matmul(acc[:], lhsT=r[:], rhs=ones[:], start=True, stop=True)
            res = pool.tile([1, 1], f32)
            nc.vector.tensor_scalar(
                out=res[:], in0=acc[:], scalar1=0.0, scalar2=None,
                op0=mybir.AluOpType.add,
            )
            nc.sync.dma_start(out_d[:], res[:])

    nc.compile()
    return nc


def _get_program():
    global _PROGRAM
    if _PROGRAM is None:
        _PROGRAM = _build_program(debug_g=DEBUG_G)
    return _PROGRAM


def _pack_core(i0_sl, i1_sl):
    """Build the [P, FV] i32 flat-index tile for one core's intervals."""
    iv = np.stack([i0_sl, i1_sl])  # [2, B_LOC, C, N, 2, 2]
    r_b = iv[..., 0, 0].reshape(-1).astype(np.int64)
    c_b = iv[..., 0, 1].reshape(-1).astype(np.int64)
    r_d = iv[..., 1, 0].reshape(-1).astype(np.int64)
    c_d = iv[..., 1, 1].reshape(-1).astype(np.int64)
    fb = (_IMGBASE + r_b * W + c_b).astype(np.int32)
    fd = (_IMGBASE + r_d * W + c_d).astype(np.int32)
    idx = np.empty((P, FV), dtype=np.int32)
    idx[_IB_P, _IB_F] = fb
    idx[_ID_P, _ID_F] = fd
    return idx


def kernel(prediction, intervals_comp_0, intervals_comp_1):
    global _LAST_RESULTS
    from concourse.bass_utils import run_bass_kernel_spmd

    nc = _get_program()

    prediction = np.asarray(prediction, dtype=np.float32)
    i0 = np.asarray(intervals_comp_0, dtype=np.int32)
    i1 = np.asarray(intervals_comp_1, dtype=np.int32)

    in_maps = []
    for mcore in range(N_CORES):
        sl = slice(mcore * B_LOC, (mcore + 1) * B_LOC)
        in_maps.append(
            {
                "pred": np.ascontiguousarray(prediction[sl]).reshape(-1),
                "idx": _pack_core(i0[sl], i1[sl]),
                "wts": _WTS,
            }
        )

    results = run_bass_kernel_spmd(
        nc, in_maps, list(range(N_CORES)), trace=TRACE
    )
    _LAST_RESULTS = results
    total = sum(float(r["out"][0, 0]) for r in results.results)
    total += N_CORES * _CONST
    return np.array(total, dtype=np.float32)


# revision 16
# speedup vs baseline: 1.4593x; 1.0193x over previous
"""BirthDeathIntervalLoss on 8 Trainium2 NeuronCores.

Strategy: the loss reads only 2*B*C*N*2 = 32768 scattered elements of the
512x512 prediction maps.  Data-parallel over batch: each core handles 4
batches (4096 gathered values).  Per core the device program is:

  1. one HWDGE DMA brings host-precomputed flat gather indices
     ([128, 32] i32) into SBUF; the per-pair weights ([NCALLS, PAIRS_ROW]
     f32) ride the scalar engine's ring in parallel,
  2. NCALLS indirect DMAs (SWDGE q0) gather the prediction values row by
     row.  The SWDGE ring retires descriptors serially at ~4ns each
     (measured; the data transfers themselves batch 64 payloads per
     packet and finish in ~1.5us), so the ~17us wall is set by 4096
     descriptors regardless of call structure; splitting into several
     calls lets descriptor generation and earlier calls' retirement
     pipeline, and a small final call shortens the completion tail.
     A gather call can only write ONE partition row (descriptor count =
     dest free size; offsets are consumed partition-fastest).
  3. the vector engine computes w*(birth-death)^2 on the [NCALLS, .] rows
     (births in the first half of each row, deaths in the second half, so
     every DVE operand is unit-stride), reduces along the free axis,
  4. the PE collapses the NCALLS partitions via ones^T matmul and one
     HWDGE DMA writes the 4-byte scalar out ([128,1]-style outputs cost
     ~8us of completion-semaphore crawl).

The host sums 8 per-core scalars and adds the closed-form constant
(this is the data-parallel all-reduce of the scalar loss).

Masked-mean algebra (unchanged from the reference):
  loss = sum_m w_m * (birth_m - death_m)^2 + B * sum_s a_s*BETA*cnt_s / C
  w_m  = a_s * (-BETA/good_s[c] if n < good_s[c] else (1-BETA)/(N-good_s[c])) / C
with a_0 = ALPHA, a_1 = 1-ALPHA, cnt_s = #{c : good_s[c] > 0}.

Descriptor walk (HW-calibrated): call c consumes offset columns
[c*CPC, (c+1)*CPC) of the [128, 32] idx tile partition-fastest — desc j
reads idx[j % 128, c*CPC + j // 128] — and writes g[c, j].
"""

import numpy as np

# ---- problem constants (hardcoded per harness contract) ----
B, C, H, W, N = 32, 4, 512, 512, 64
GOOD = np.array([[1, 2, 1, 3], [1, 0, 2, 1]], dtype=np.int64)  # [set, class]
ALPHA = 0.5
BETA = 0.5
N_CORES = 8
B_LOC = B // N_CORES  # 4 batches per core

PRED_LOC = B_LOC * C * H * W          # 4,194,304 f32 per core
N_PAIRS = 2 * B_LOC * C * N           # 2048 (birth,death) pairs per core
N_VALS = 2 * N_PAIRS                  # 4096 gathered values per core

P = 128                               # partitions
FV = N_VALS // P                      # 32 offset columns

NCALLS = 8                            # gather calls (rows of g)
CPC = FV // NCALLS                    # offset columns per call
ROWLEN = N_VALS // NCALLS             # values per row
HALF = ROWLEN // 2                    # pairs per row
SCRATCH = 16384                       # SWDGE descriptor ring bytes


def _host_constants():
    """Natural-order pair weights w[m] and the per-core additive constant."""
    a = np.array([ALPHA, 1.0 - ALPHA])
    m = np.arange(N_PAIRS)
    s = m // (B_LOC * C * N)
    cc = (m // N) % C
    n = m % N
    g = GOOD[s, cc]
    w = np.where(
        n < g,
        -a[s] * BETA / np.maximum(g, 1) / C,
        a[s] * (1.0 - BETA) / (N - g) / C,
    ).astype(np.float32)
    cnt = (GOOD > 0).sum(axis=1)  # per set
    const_per_core = float((a * BETA * cnt / C).sum() * B_LOC)
    return w, const_per_core


_W_NAT, _CONST = _host_constants()

# pair m -> (call c, pair slot q); birth desc j=q, death desc j=HALF+q of call c
_MC = np.arange(N_PAIRS) // HALF
_MQ = np.arange(N_PAIRS) % HALF

_WTS = _W_NAT.reshape(NCALLS, HALF).copy()  # wts[c, q] = w of pair c*HALF+q

# offset slot of desc j in call c: idx[j % P, c*CPC + j // P]
_JB = _MQ            # birth desc id within call
_JD = HALF + _MQ     # death desc id within call
_IB_P = _JB % P
_IB_F = _MC * CPC + _JB // P
_ID_P = _JD % P
_ID_F = _MC * CPC + _JD // P

# per-pair image base
_MB = (np.arange(N_PAIRS) // (C * N)) % B_LOC
_MCC = (np.arange(N_PAIRS) // N) % C
_IMGBASE = ((_MB * C + _MCC) * (H * W)).astype(np.int64)  # per pair

_PROGRAM = None
_LAST_RESULTS = None  # BassKernelResults of the most recent run (for test.py)
TRACE = False
DEBUG_G = False  # build with an extra DMA dumping the gathered rows


def _build_program(debug_g=False):
    from concourse import bacc, mybir
    import concourse.bass as bass
    import concourse.tile as tile

    f32 = mybir.dt.float32
    i32 = mybir.dt.int32

    nc = bacc.Bacc(
        "TRN2",
        target_bir_lowering=False,
        debug=False,
        dynamic_dma_scratch_size=SCRATCH,
    )

    pred_d = nc.dram_tensor("pred", [PRED_LOC], f32, kind="ExternalInput")
    idx_d = nc.dram_tensor("idx", [P, FV], i32, kind="ExternalInput")
    wts_d = nc.dram_tensor("wts", [NCALLS, HALF], f32, kind="ExternalInput")
    out_d = nc.dram_tensor("out", [1, 1], f32, kind="ExternalOutput")
    if debug_g:
        g_d = nc.dram_tensor("gdump", [NCALLS, ROWLEN], f32, kind="ExternalOutput")

    with tile.TileContext(nc) as tc:
        with (
            tc.tile_pool(name="sb", bufs=1) as pool,
            tc.tile_pool(name="ps", bufs=1, space="PSUM") as psp,
        ):
            idx = pool.tile([P, FV], i32)
            nc.sync.dma_start(idx[:], idx_d[:])
            wts = pool.tile([NCALLS, HALF], f32)
            nc.scalar.dma_start(wts[:], wts_d[:])
            ones = pool.tile([NCALLS, 1], f32)
            nc.vector.memset(ones[:], 1.0)

            g = pool.tile([NCALLS, ROWLEN], f32)
            src = pred_d.ap().rearrange("(a f) -> a f", a=1)
            for c in range(NCALLS):
                nc.gpsimd.indirect_dma_start(
                    out=g[c : c + 1, :].rearrange("a (f one) -> a f one", one=1),
                    out_offset=None,
                    in_=src,
                    in_offset=bass.IndirectOffsetOnAxis(
                        ap=idx[:, c * CPC : (c + 1) * CPC], axis=1
                    ),
                )
            if debug_g:
                nc.scalar.dma_start(g_d[:], g[:])

            d = pool.tile([NCALLS, HALF], f32)
            nc.vector.tensor_tensor(
                out=d[:], in0=g[:, 0:HALF], in1=g[:, HALF:ROWLEN],
                op=mybir.AluOpType.subtract,
            )
            d2 = pool.tile([NCALLS, HALF], f32)
            nc.vector.tensor_tensor(
                out=d2[:], in0=d[:], in1=d[:], op=mybir.AluOpType.mult
            )
            dw = pool.tile([NCALLS, HALF], f32)
            nc.vector.tensor_tensor(
                out=dw[:], in0=d2[:], in1=wts[:], op=mybir.AluOpType.mult
            )
            r = pool.tile([NCALLS, 1], f32)
            nc.vector.reduce_sum(out=r[:], in_=dw[:], axis=mybir.AxisListType.X)
            # lhsT=ones so the PE weight load doesn't wait on the reduce
            acc = psp.tile([1, 1], f32)
            nc.tensor.matmul(acc[:], lhsT=ones[:], rhs=r[:], start=True, stop=True)
            res = pool.tile([1, 1], f32)
            nc.vector.tensor_scalar(
                out=res[:], in0=acc[:], scalar1=0.0, scalar2=None,
                op0=mybir.AluOpType.add,
            )
            nc.sync.dma_start(out_d[:], res[:])

    nc.compile()
    return nc


def _get_program():
    global _PROGRAM
    if _PROGRAM is None:
        _PROGRAM = _build_program(debug_g=DEBUG_G)
    return _PROGRAM


def _pack_core(i0_sl, i1_sl):
    """Build the [P, FV] i32 flat-index tile for one core's intervals."""
    iv = np.stack([i0_sl, i1_sl])  # [2, B_LOC, C, N, 2, 2]
    r_b = iv[..., 0, 0].reshape(-1).astype(np.int64)
    c_b = iv[..., 0, 1].reshape(-1).astype(np.int64)
    r_d = iv[..., 1, 0].reshape(-1).astype(np.int64)
    c_d = iv[..., 1, 1].reshape(-1).astype(np.int64)
    fb = (_IMGBASE + r_b * W + c_b).astype(np.int32)
    fd = (_IMGBASE + r_d * W + c_d).astype(np.int32)
    idx = np.empty((P, FV), dtype=np.int32)
    idx[_IB_P, _IB_F] = fb
    idx[_ID_P, _ID_F] = fd
    return idx


def kernel(prediction, intervals_comp_0, intervals_comp_1):
    global _LAST_RESULTS
    from concourse.bass_utils import run_bass_kernel_spmd

    nc = _get_program()

    prediction = np.asarray(prediction, dtype=np.float32)
    i0 = np.asarray(intervals_comp_0, dtype=np.int32)
    i1 = np.asarray(intervals_comp_1, dtype=np.int32)

    in_maps = []
    for mcore in range(N_CORES):
        sl = slice(mcore * B_LOC, (mcore + 1) * B_LOC)
        in_maps.append(
            {
                "pred": np.ascontiguousarray(prediction[sl]).reshape(-1),
                "idx": _pack_core(i0[sl], i1[sl]),
                "wts": _WTS,
            }
        )

    results = run_bass_kernel_spmd(
        nc, in_maps, list(range(N_CORES)), trace=TRACE
    )
    _LAST_RESULTS = results
    total = sum(float(r["out"][0, 0]) for r in results.results)
    total += N_CORES * _CONST
    return np.array(total, dtype=np.float32)


# revision 17
# speedup vs baseline: 1.4647x; 1.0037x over previous
"""BirthDeathIntervalLoss on 8 Trainium2 NeuronCores.

Strategy: the loss reads only 2*B*C*N*2 = 32768 scattered elements of the
512x512 prediction maps.  Data-parallel over batch: each core handles 4
batches (4096 gathered values).  Per core the device program is:

  1. one HWDGE DMA brings host-precomputed flat gather indices
     ([128, 32] i32) into SBUF; the per-pair weights ([NCALLS, PAIRS_ROW]
     f32) ride the scalar engine's ring in parallel,
  2. NCALLS indirect DMAs (SWDGE q0) gather the prediction values row by
     row.  The SWDGE ring retires descriptors serially at ~4ns each
     (measured; the data transfers themselves batch 64 payloads per
     packet and finish in ~1.5us), so the ~17us wall is set by 4096
     descriptors regardless of call structure; splitting into 8 even
     calls lets descriptor generation (~1.7us/call) pipeline under the
     retirement so the wall stays at ~17us (1-2 calls measure 23-24us).
     A gather call can only write ONE partition row (descriptor count =
     dest free size; offsets are consumed partition-fastest).
  3. the vector engine computes w*(birth-death)^2 on the [NCALLS, .] rows
     (births in the first half of each row, deaths in the second half, so
     every DVE operand is unit-stride), reduces along the free axis,
  4. the PE collapses the NCALLS partitions via ones^T matmul and one
     HWDGE DMA writes the 4-byte scalar out ([128,1]-style outputs cost
     ~8us of completion-semaphore crawl).

The host sums 8 per-core scalars and adds the closed-form constant
(this is the data-parallel all-reduce of the scalar loss).

Masked-mean algebra (unchanged from the reference):
  loss = sum_m w_m * (birth_m - death_m)^2 + B * sum_s a_s*BETA*cnt_s / C
  w_m  = a_s * (-BETA/good_s[c] if n < good_s[c] else (1-BETA)/(N-good_s[c])) / C
with a_0 = ALPHA, a_1 = 1-ALPHA, cnt_s = #{c : good_s[c] > 0}.

Descriptor walk (HW-calibrated): call c consumes offset columns
[c*CPC, (c+1)*CPC) of the [128, 32] idx tile partition-fastest — desc j
reads idx[j % 128, c*CPC + j // 128] — and writes g[c, j].
"""

import numpy as np

# ---- problem constants (hardcoded per harness contract) ----
B, C, H, W, N = 32, 4, 512, 512, 64
GOOD = np.array([[1, 2, 1, 3], [1, 0, 2, 1]], dtype=np.int64)  # [set, class]
ALPHA = 0.5
BETA = 0.5
N_CORES = 8
B_LOC = B // N_CORES  # 4 batches per core

PRED_LOC = B_LOC * C * H * W          # 4,194,304 f32 per core
N_PAIRS = 2 * B_LOC * C * N           # 2048 (birth,death) pairs per core
N_VALS = 2 * N_PAIRS                  # 4096 gathered values per core

P = 128                               # partitions
FV = N_VALS // P                      # 32 offset columns

NCALLS = 8                            # gather calls (rows of g)
CPC = FV // NCALLS                    # offset columns per call
ROWLEN = N_VALS // NCALLS             # values per row
HALF = ROWLEN // 2                    # pairs per row
SCRATCH = 16384                       # SWDGE descriptor ring bytes


def _host_constants():
    """Natural-order pair weights w[m] and the per-core additive constant."""
    a = np.array([ALPHA, 1.0 - ALPHA])
    m = np.arange(N_PAIRS)
    s = m // (B_LOC * C * N)
    cc = (m // N) % C
    n = m % N
    g = GOOD[s, cc]
    w = np.where(
        n < g,
        -a[s] * BETA / np.maximum(g, 1) / C,
        a[s] * (1.0 - BETA) / (N - g) / C,
    ).astype(np.float32)
    cnt = (GOOD > 0).sum(axis=1)  # per set
    const_per_core = float((a * BETA * cnt / C).sum() * B_LOC)
    return w, const_per_core


_W_NAT, _CONST = _host_constants()

# pair m -> (call c, pair slot q); birth desc j=q, death desc j=HALF+q of call c
_MC = np.arange(N_PAIRS) // HALF
_MQ = np.arange(N_PAIRS) % HALF

_WTS = _W_NAT.reshape(NCALLS, HALF).copy()  # wts[c, q] = w of pair c*HALF+q

# offset slot of desc j in call c: idx[j % P, c*CPC + j // P]
_JB = _MQ            # birth desc id within call
_JD = HALF + _MQ     # death desc id within call
_IB_P = _JB % P
_IB_F = _MC * CPC + _JB // P
_ID_P = _JD % P
_ID_F = _MC * CPC + _JD // P

# per-pair image base
_MB = (np.arange(N_PAIRS) // (C * N)) % B_LOC
_MCC = (np.arange(N_PAIRS) // N) % C
_IMGBASE = ((_MB * C + _MCC) * (H * W)).astype(np.int64)  # per pair

_PROGRAM = None
_LAST_RESULTS = None  # BassKernelResults of the most recent run (for test.py)
TRACE = False
DEBUG_G = False  # build with an extra DMA dumping the gathered rows


def _build_program(debug_g=False):
    from concourse import bacc, mybir
    import concourse.bass as bass
    import concourse.tile as tile

    f32 = mybir.dt.float32
    i32 = mybir.dt.int32

    nc = bacc.Bacc(
        "TRN2",
        target_bir_lowering=False,
        debug=False,
        dynamic_dma_scratch_size=SCRATCH,
    )

    pred_d = nc.dram_tensor("pred", [PRED_LOC], f32, kind="ExternalInput")
    idx_d = nc.dram_tensor("idx", [P, FV], i32, kind="ExternalInput")
    wts_d = nc.dram_tensor("wts", [NCALLS, HALF], f32, kind="ExternalInput")
    out_d = nc.dram_tensor("out", [1, 1], f32, kind="ExternalOutput")
    if debug_g:
        g_d = nc.dram_tensor("gdump", [NCALLS, ROWLEN], f32, kind="ExternalOutput")

    with tile.TileContext(nc) as tc:
        with (
            tc.tile_pool(name="sb", bufs=1) as pool,
            tc.tile_pool(name="ps", bufs=1, space="PSUM") as psp,
        ):
            idx = pool.tile([P, FV], i32)
            nc.sync.dma_start(idx[:], idx_d[:])
            wts = pool.tile([NCALLS, HALF], f32)
            nc.scalar.dma_start(wts[:], wts_d[:])
            ones = pool.tile([NCALLS, 1], f32)
            nc.vector.memset(ones[:], 1.0)

            g = pool.tile([NCALLS, ROWLEN], f32)
            src = pred_d.ap().rearrange("(a f) -> a f", a=1)
            for c in range(NCALLS):
                nc.gpsimd.indirect_dma_start(
                    out=g[c : c + 1, :].rearrange("a (f one) -> a f one", one=1),
                    out_offset=None,
                    in_=src,
                    in_offset=bass.IndirectOffsetOnAxis(
                        ap=idx[:, c * CPC : (c + 1) * CPC], axis=1
                    ),
                )
            if debug_g:
                nc.scalar.dma_start(g_d[:], g[:])

            d = pool.tile([NCALLS, HALF], f32)
            nc.vector.tensor_tensor(
                out=d[:], in0=g[:, 0:HALF], in1=g[:, HALF:ROWLEN],
                op=mybir.AluOpType.subtract,
            )
            d2 = pool.tile([NCALLS, HALF], f32)
            nc.vector.tensor_tensor(
                out=d2[:], in0=d[:], in1=d[:], op=mybir.AluOpType.mult
            )
            dw = pool.tile([NCALLS, HALF], f32)
            nc.vector.tensor_tensor(
                out=dw[:], in0=d2[:], in1=wts[:], op=mybir.AluOpType.mult
            )
            r = pool.tile([NCALLS, 1], f32)
            nc.vector.reduce_sum(out=r[:], in_=dw[:], axis=mybir.AxisListType.X)
            # lhsT=ones so the PE weight load doesn't wait on the reduce
            acc = psp.tile([1, 1], f32)
            nc.tensor.matmul(acc[:], lhsT=ones[:], rhs=r[:], start=True, stop=True)
            res = pool.tile([1, 1], f32)
            nc.vector.tensor_scalar(
                out=res[:], in0=acc[:], scalar1=0.0, scalar2=None,
                op0=mybir.AluOpType.add,
            )
            nc.sync.dma_start(out_d[:], res[:])

    nc.compile()
    return nc


def _get_program():
    global _PROGRAM
    if _PROGRAM is None:
        _PROGRAM = _build_program(debug_g=DEBUG_G)
    return _PROGRAM


def _pack_core(i0_sl, i1_sl):
    """Build the [P, FV] i32 flat-index tile for one core's intervals."""
    iv = np.stack([i0_sl, i1_sl])  # [2, B_LOC, C, N, 2, 2]
    r_b = iv[..., 0, 0].reshape(-1).astype(np.int64)
    c_b = iv[..., 0, 1].reshape(-1).astype(np.int64)
    r_d = iv[..., 1, 0].reshape(-1).astype(np.int64)
    c_d = iv[..., 1, 1].reshape(-1).astype(np.int64)
    fb = (_IMGBASE + r_b * W + c_b).astype(np.int32)
    fd = (_IMGBASE + r_d * W + c_d).astype(np.int32)
    idx = np.empty((P, FV), dtype=np.int32)
    idx[_IB_P, _IB_F] = fb
    idx[_ID_P, _ID_F] = fd
    return idx


def kernel(prediction, intervals_comp_0, intervals_comp_1):
    global _LAST_RESULTS
    from concourse.bass_utils import run_bass_kernel_spmd

    nc = _get_program()

    prediction = np.asarray(prediction, dtype=np.float32)
    i0 = np.asarray(intervals_comp_0, dtype=np.int32)
    i1 = np.asarray(intervals_comp_1, dtype=np.int32)

    in_maps = []
    for mcore in range(N_CORES):
        sl = slice(mcore * B_LOC, (mcore + 1) * B_LOC)
        in_maps.append(
            {
                "pred": np.ascontiguousarray(prediction[sl]).reshape(-1),
                "idx": _pack_core(i0[sl], i1[sl]),
                "wts": _WTS,
            }
        )

    results = run_bass_kernel_spmd(
        nc, in_maps, list(range(N_CORES)), trace=TRACE
    )
    _LAST_RESULTS = results
    total = sum(float(r["out"][0, 0]) for r in results.results)
    total += N_CORES * _CONST
    return np.array(total, dtype=np.float32)
